# revision 1
# baseline (speedup 1.0000x reference)
"""InfoVAE loss kernel for Trainium2, data-parallel over batch on 8 NeuronCores.

Reference computation (see problem spec):
    recons_loss = mean((recons - x)^2)                    recons/x: [4096, 3, 64, 64]
    mmd  = km(pz,pz) + km(z,z) - 2*km(pz,z)               z/pz:     [4096, 128]
           where km(a,b) = mean_ij exp(-(|a_i-b_j|^2/D)/sigma), sigma = 2*D*z_var
    kld  = mean_n(-0.5 * sum_d(1 + lv - mu^2 - exp(lv)))
    loss = 5*recons_loss + 1.5*(1/N)*kld + 98.5/(N*(N-1))*mmd
    returns (loss, recons_loss, mmd, -kld)

Sharding: each core owns a 512-row block of the batch. The RBF kernel blocks are
computed as block-rows vs the full gathered z/prior_z (replicated, 2 MB each).
Per-core partial sums come back as small per-partition accumulator tiles; the
final (tiny) reduction is done on host in float64.

RBF assembly on device: arg_ij = a_i.b_j/32768 - |a_i|^2/65536 - |b_j|^2/65536.
 - a_i.b_j/32768 : PE matmul with the block lhsT pre-scaled by 2^-15 (exact).
 - -|b_j|^2/65536: a K=1 accumulating matmul (ones outer-product row term).
 - -|a_i|^2/65536: per-partition bias of the ACT Exp instruction.
ACT's fused accum_out gives the per-partition running sums for free.
"""

import numpy as np

N = 4096
D = 128
NCORES = 8
ROWS = N // NCORES            # 512 rows per core
IMG_F = 3 * 64 * 64           # 12288
P = 128
T_ROW = ROWS // P             # 4 row tiles per core
MSE_CHUNK = 2048
MSE_NCH = IMG_F // MSE_CHUNK  # 6
JG = 1024                     # psum group width for the rbf matmuls
NJG = N // JG                 # 4 j-groups
Z_VAR = 2.0
SIGMA = 2.0 * D * Z_VAR       # 512
INV_2S = 1.0 / (D * SIGMA / 2.0)   # 1/32768 (exact power of two)
INV_S = 1.0 / (D * SIGMA)          # 1/65536

_CACHE = {}


def _build():
    import concourse.bass as bass
    import concourse.tile as tile
    from concourse import bacc, mybir

    f32 = mybir.dt.float32
    AF = mybir.ActivationFunctionType
    ALU = mybir.AluOpType
    AX = mybir.AxisListType

    nc = bacc.Bacc("TRN2", target_bir_lowering=False, debug=False,
                   num_devices=NCORES)

    r_blk = nc.dram_tensor("r_blk", [ROWS, IMG_F], f32, kind="ExternalInput").ap()
    x_blk = nc.dram_tensor("x_blk", [ROWS, IMG_F], f32, kind="ExternalInput").ap()
    z_full = nc.dram_tensor("z_full", [N, D], f32, kind="ExternalInput").ap()
    pz_full = nc.dram_tensor("pz_full", [N, D], f32, kind="ExternalInput").ap()
    z_blk = nc.dram_tensor("z_blk", [ROWS, D], f32, kind="ExternalInput").ap()
    pz_blk = nc.dram_tensor("pz_blk", [ROWS, D], f32, kind="ExternalInput").ap()
    mu_blk = nc.dram_tensor("mu_blk", [ROWS, D], f32, kind="ExternalInput").ap()
    lv_blk = nc.dram_tensor("lv_blk", [ROWS, D], f32, kind="ExternalInput").ap()
    ident = nc.dram_tensor("ident", [P, P], f32, kind="ExternalInput").ap()

    NMSE = T_ROW * MSE_NCH            # 24 accum columns
    NMMD = 3 * T_ROW * NJG            # 48 accum columns
    mse_out = nc.dram_tensor("mse_acc", [P, NMSE], f32, kind="ExternalOutput").ap()
    mmd_out = nc.dram_tensor("mmd_acc", [P, NMMD], f32, kind="ExternalOutput").ap()
    kld_out = nc.dram_tensor("kld_acc", [P, 4], f32, kind="ExternalOutput").ap()

    with tile.TileContext(nc) as tc:
        with (
            tc.tile_pool(name="consts", bufs=1) as consts,
            tc.tile_pool(name="nat", bufs=1) as nat,
            tc.tile_pool(name="stream", bufs=3) as stream,
            tc.tile_pool(name="dpool", bufs=2) as dpool,
            tc.tile_pool(name="tstage", bufs=2) as tstage,
            tc.tile_pool(name="scratch", bufs=2) as scratch,
            tc.tile_pool(name="acc", bufs=1) as accp,
            tc.tile_pool(name="psmm", bufs=3, space="PSUM") as psmm,
            tc.tile_pool(name="pstr", bufs=2, space="PSUM") as pstr,
        ):
            # ---- constants / small setup ----
            ident_sb = consts.tile([P, P], f32)
            nc.sync.dma_start(out=ident_sb[:], in_=ident)
            ones_row = consts.tile([1, P], f32)
            nc.vector.memset(ones_row[:], 1.0)
            ones_col = consts.tile([P, 1], f32)
            nc.vector.memset(ones_col[:], 1.0)
            negs_col = consts.tile([P, 1], f32)       # -1/65536 column for norm matmuls
            nc.vector.memset(negs_col[:], -INV_S)

            # accumulators
            mse_cols = accp.tile([P, NMSE], f32)
            mmd_cols = accp.tile([P, NMMD], f32)
            kld_cols = accp.tile([P, 4], f32)
            nc.vector.memset(kld_cols[:, 3:4], 0.0)

            zv = z_full.rearrange("(t p) d -> p t d", p=P)
            pv = pz_full.rearrange("(t p) d -> p t d", p=P)

            # block rows natural (for bias norms + block transpose)
            zb_nat = nat.tile([P, T_ROW, D], f32)
            pb_nat = nat.tile([P, T_ROW, D], f32)
            nc.sync.dma_start(out=zb_nat[:], in_=z_blk.rearrange("(t p) d -> p t d", p=P))
            nc.sync.dma_start(out=pb_nat[:], in_=pz_blk.rearrange("(t p) d -> p t d", p=P))

            # ---- transpose z/pz to [d, j] layout via PE (staged loads) ----
            zT = consts.tile([P, N], f32)
            pzT = consts.tile([P, N], f32)
            for (view, dst) in ((zv, zT), (pv, pzT)):
                for g in range(4):                # stage 8 row-tiles (1 MB) at a time
                    st = tstage.tile([P, 8, D], f32, tag="tst")
                    nc.sync.dma_start(out=st[:, 0:4, :], in_=view[:, g * 8:g * 8 + 4, :])
                    nc.sync.dma_start(out=st[:, 4:8, :], in_=view[:, g * 8 + 4:g * 8 + 8, :])
                    for gg in range(2):           # 4 transposes per psum tile
                        tp = pstr.tile([P, 512], f32, tag="tr")
                        for k in range(4):
                            nc.tensor.transpose(tp[:, k * P:(k + 1) * P],
                                                st[:, gg * 4 + k, :], ident_sb[:])
                        col = (g * 8 + gg * 4) * P
                        nc.vector.tensor_copy(dst[:, col:col + 512], tp[:])

            # block transposed & pre-scaled by 1/32768 (exact pow2)
            zbTs = consts.tile([P, ROWS], f32)
            pbTs = consts.tile([P, ROWS], f32)
            for (src, dst) in ((zb_nat, zbTs), (pb_nat, pbTs)):
                tp = pstr.tile([P, 512], f32, tag="tr")
                for t in range(T_ROW):
                    nc.tensor.transpose(tp[:, t * P:(t + 1) * P], src[:, t, :],
                                        ident_sb[:])
                nc.vector.tensor_scalar_mul(dst[:], tp[:], INV_2S)

            # ---- column norm rows: negnorm[j] = -|b_j|^2/65536, laid [1, N] ----
            nn_z = consts.tile([1, N], f32)
            nn_pz = consts.tile([1, N], f32)
            for (srcT, dst) in ((zT, nn_z), (pzT, nn_pz)):
                for c in range(N // 512):
                    sq = scratch.tile([P, 512], f32, tag="sq")
                    nc.vector.tensor_mul(sq[:], srcT[:, c * 512:(c + 1) * 512],
                                         srcT[:, c * 512:(c + 1) * 512])
                    npm = pstr.tile([P, 512], f32, tag="tr")
                    nc.tensor.matmul(npm[0:1, :], lhsT=negs_col[:], rhs=sq[:],
                                     start=True, stop=True)
                    nc.vector.tensor_copy(dst[0:1, c * 512:(c + 1) * 512],
                                          npm[0:1, :])

            # ---- row-bias tiles: bias_a[:, t] = -|a_i|^2/65536 for block rows ----
            bias_z = consts.tile([P, T_ROW], f32)
            bias_pz = consts.tile([P, T_ROW], f32)
            for (src, dst) in ((zb_nat, bias_z), (pb_nat, bias_pz)):
                for t in range(T_ROW):
                    sq2 = scratch.tile([P, D], f32, tag="sq2")
                    # Square(x/256) = x^2/65536 (scale is an exact pow2)
                    nc.scalar.activation(out=sq2[:], in_=src[:, t, :],
                                         func=AF.Square, scale=1.0 / 256.0,
                                         accum_out=dst[:, t:t + 1])
                nc.vector.tensor_scalar_mul(dst[:], dst[:], -1.0)

            # ---- KLD block terms ----
            mu_t = nat.tile([P, T_ROW, D], f32)
            lv_t = nat.tile([P, T_ROW, D], f32)
            nc.sync.dma_start(out=mu_t[:], in_=mu_blk.rearrange("(t p) d -> p t d", p=P))
            nc.sync.dma_start(out=lv_t[:], in_=lv_blk.rearrange("(t p) d -> p t d", p=P))
            ksc = scratch.tile([P, T_ROW, D], f32, tag="ksc")
            nc.vector.tensor_reduce(kld_cols[:, 0:1], lv_t[:], axis=AX.XY,
                                    op=ALU.add)
            nc.scalar.activation(out=ksc[:], in_=mu_t[:], func=AF.Square,
                                 accum_out=kld_cols[:, 1:2])
            ksc2 = scratch.tile([P, T_ROW, D], f32, tag="ksc")
            nc.scalar.activation(out=ksc2[:], in_=lv_t[:], func=AF.Exp,
                                 accum_out=kld_cols[:, 2:3])

            # ---- interleaved main loops: MMD rbf blocks + MSE stream ----
            pairs = [(pbTs, pzT, nn_pz, bias_pz),   # k(pz, pz)
                     (zbTs, zT, nn_z, bias_z),      # k(z, z)
                     (pbTs, zT, nn_z, bias_pz)]     # k(pz, z)

            rv = r_blk.rearrange("(t p) f -> p t f", p=P)
            xv = x_blk.rearrange("(t p) f -> p t f", p=P)

            def emit_mse(k):
                t, c = divmod(k, MSE_NCH)
                rt = stream.tile([P, MSE_CHUNK], f32, tag="rt")
                xt = stream.tile([P, MSE_CHUNK], f32, tag="xt")
                # split each chunk load 4 ways so multiple DMA queues fill one
                # buffer concurrently (per-queue bw is ~27 GiB/s)
                w = MSE_CHUNK // 4
                for h in range(4):
                    lo = c * MSE_CHUNK + h * w
                    nc.sync.dma_start(out=rt[:, h * w:(h + 1) * w],
                                      in_=rv[:, t, lo:lo + w])
                    nc.sync.dma_start(out=xt[:, h * w:(h + 1) * w],
                                      in_=xv[:, t, lo:lo + w])
                dt = dpool.tile([P, MSE_CHUNK], f32)
                nc.vector.tensor_sub(dt[:], rt[:], xt[:])
                sc = scratch.tile([P, MSE_CHUNK], f32, tag="msq")
                nc.scalar.activation(out=sc[:], in_=dt[:], func=AF.Square,
                                     accum_out=mse_cols[:, k:k + 1])

            def emit_mmd(k):
                pi, rem = divmod(k, T_ROW * NJG)
                t, jg = divmod(rem, NJG)
                aTs, bT, nn_b, bias_a = pairs[pi]
                ps = psmm.tile([P, JG], f32)
                for jc in range(JG // 512):
                    j = jg * (JG // 512) + jc
                    nc.tensor.matmul(ps[:, jc * 512:(jc + 1) * 512],
                                     lhsT=aTs[:, t * P:(t + 1) * P],
                                     rhs=bT[:, j * 512:(j + 1) * 512],
                                     start=True, stop=False)
                    nc.tensor.matmul(ps[:, jc * 512:(jc + 1) * 512],
                                     lhsT=ones_row[:], rhs=nn_b[0:1, j * 512:(j + 1) * 512],
                                     start=False, stop=True)
                sc = scratch.tile([P, JG], f32, tag="esc")
                nc.scalar.activation(out=sc[:], in_=ps[:], func=AF.Exp,
                                     bias=bias_a[:, t:t + 1], scale=1.0,
                                     accum_out=mmd_cols[:, k:k + 1])

            for k in range(NMMD):
                emit_mmd(k)
                if k % 2 == 0 and k // 2 < NMSE:
                    emit_mse(k // 2)

            # ---- write partials out ----
            nc.sync.dma_start(out=mse_out, in_=mse_cols[:])
            nc.sync.dma_start(out=mmd_out, in_=mmd_cols[:])
            nc.sync.dma_start(out=kld_out, in_=kld_cols[:])

    nc.compile()
    return nc


def get_nc():
    if "nc" not in _CACHE:
        _CACHE["nc"] = _build()
    return _CACHE["nc"]


def make_in_maps(recons, x, z, mu, log_var, prior_z):
    r2 = np.ascontiguousarray(recons, dtype=np.float32).reshape(N, IMG_F)
    x2 = np.ascontiguousarray(x, dtype=np.float32).reshape(N, IMG_F)
    z = np.ascontiguousarray(z, dtype=np.float32)
    pz = np.ascontiguousarray(prior_z, dtype=np.float32)
    mu = np.ascontiguousarray(mu, dtype=np.float32)
    lv = np.ascontiguousarray(log_var, dtype=np.float32)
    ident = np.eye(P, dtype=np.float32)
    maps = []
    for c in range(NCORES):
        s = slice(c * ROWS, (c + 1) * ROWS)
        maps.append({
            "r_blk": r2[s], "x_blk": x2[s],
            "z_full": z, "pz_full": pz,
            "z_blk": z[s], "pz_blk": pz[s],
            "mu_blk": mu[s], "lv_blk": lv[s],
            "ident": ident,
        })
    return maps


def combine(results):
    mse_sum = 0.0
    s_pp = s_zz = s_pz = 0.0
    kld_total = 0.0
    per_pair = T_ROW * NJG
    for res in results:
        mse_sum += np.float64(res["mse_acc"]).sum()
        m = np.float64(res["mmd_acc"])
        s_pp += m[:, 0:per_pair].sum()
        s_zz += m[:, per_pair:2 * per_pair].sum()
        s_pz += m[:, 2 * per_pair:3 * per_pair].sum()
        k = np.float64(res["kld_acc"])
        kld_total += ROWS * D + k[:, 0].sum() - k[:, 1].sum() - k[:, 2].sum()

    recons_loss = mse_sum / (N * IMG_F)
    mmd = (s_pp + s_zz - 2.0 * s_pz) / (float(N) * float(N))
    kld = -0.5 * kld_total / N
    beta, alpha, reg_w = 5.0, -0.5, 100.0
    loss = (beta * recons_loss
            + (1.0 - alpha) * (1.0 / N) * kld
            + (alpha + reg_w - 1.0) / (float(N) * (N - 1)) * mmd)
    return (np.float32(loss), np.float32(recons_loss),
            np.float32(mmd), np.float32(-kld))


def run(recons, x, z, mu, log_var, prior_z, trace=False):
    from concourse.bass_utils import run_bass_kernel_spmd
    nc = get_nc()
    in_maps = make_in_maps(recons, x, z, mu, log_var, prior_z)
    res = run_bass_kernel_spmd(nc, in_maps, list(range(NCORES)), trace=trace)
    return res


def kernel(recons, x, z, mu, log_var, prior_z):
    res = run(recons, x, z, mu, log_var, prior_z)
    return combine(res.results)



# revision 3
# speedup vs baseline: 2.9221x; 2.9221x over previous
"""InfoVAE loss kernel for Trainium2, data-parallel over batch on 8 NeuronCores.

Reference computation (see problem spec):
    recons_loss = mean((recons - x)^2)                    recons/x: [4096, 3, 64, 64]
    mmd  = km(pz,pz) + km(z,z) - 2*km(pz,z)               z/pz:     [4096, 128]
           where km(a,b) = mean_ij exp(-(|a_i-b_j|^2/D)/sigma), sigma = 2*D*z_var
    kld  = mean_n(-0.5 * sum_d(1 + lv - mu^2 - exp(lv)))
    loss = 5*recons_loss + 1.5*(1/N)*kld + 98.5/(N*(N-1))*mmd
    returns (loss, recons_loss, mmd, -kld)

MMD via factored Taylor moments instead of the N^2 pairwise kernel:
    k_ij = e^{-r_i} e^{-c_j} e^{p_ij},  r_i=|a_i|^2/2^16, c_j=|b_j|^2/2^16,
    p_ij = a_i.b_j/2^15.  |p| <~ 1e-3, so e^p = 1 + p + p^2/2 to ~1e-10:
      sum_ij k = S_A*S_B + (w_A.w_B)/2^15 + <G_A, G_B>/2^31
    with per-tensor weighted moments S = sum_i e^{-r_i}, w = sum_i e^{-r_i} a_i,
    G = sum_i e^{-r_i} a_i a_i^T.  All moments are additive over row blocks, so
    each core computes them for its own 512-row block; the host sums the 8
    block moments and assembles the three pair sums in float64.  Validated vs
    the f64 ground truth: mmd rel err ~1e-7 (the fp32 jax reference itself sits
    7.2e-3 from the f64 value; tolerance is 2e-2).

This removes every N^2 term: the kernel is a pure memory-bound stream of
recons/x (50 MB/core) with the tiny moment/KLD work hidden under the DMA.
"""

import numpy as np

N = 4096
D = 128
NCORES = 8
ROWS = N // NCORES            # 512 rows per core
IMG_F = 3 * 64 * 64           # 12288
P = 128
T_ROW = ROWS // P             # 4 row tiles per core
MSE_CHUNK = 2048
MSE_NCH = IMG_F // MSE_CHUNK  # 6
NMSE = T_ROW * MSE_NCH        # 24 accum columns
INV_2S = 1.0 / 2.0 ** 15
INV_S = 1.0 / 2.0 ** 16

# small_out column map
C_MSE = 0                     # 0:24   mse partial sums
C_LV = 24                     # sum(log_var)
C_MU2 = 25                    # sum(mu^2)
C_ELV = 26                    # sum(exp(log_var))
C_SZ = 28                     # S_z partial (per-partition)
C_SPZ = 29                    # S_pz partial
NSMALL = 30
# gw_out column map: [G_z | w_z | G_pz | w_pz]
GW_W = D + 1                  # 129 columns per tensor
NGW = 2 * GW_W                # 258

_CACHE = {}


def _build():
    import concourse.bass as bass
    import concourse.tile as tile
    from concourse import bacc, mybir

    f32 = mybir.dt.float32
    AF = mybir.ActivationFunctionType
    ALU = mybir.AluOpType
    AX = mybir.AxisListType

    nc = bacc.Bacc("TRN2", target_bir_lowering=False, debug=False,
                   num_devices=NCORES)

    r_blk = nc.dram_tensor("r_blk", [ROWS, IMG_F], f32, kind="ExternalInput").ap()
    x_blk = nc.dram_tensor("x_blk", [ROWS, IMG_F], f32, kind="ExternalInput").ap()
    z_blk = nc.dram_tensor("z_blk", [ROWS, D], f32, kind="ExternalInput").ap()
    pz_blk = nc.dram_tensor("pz_blk", [ROWS, D], f32, kind="ExternalInput").ap()
    mu_blk = nc.dram_tensor("mu_blk", [ROWS, D], f32, kind="ExternalInput").ap()
    lv_blk = nc.dram_tensor("lv_blk", [ROWS, D], f32, kind="ExternalInput").ap()

    small_out = nc.dram_tensor("small_out", [P, NSMALL], f32,
                               kind="ExternalOutput").ap()
    gw_out = nc.dram_tensor("gw_out", [P, NGW], f32, kind="ExternalOutput").ap()

    with tile.TileContext(nc) as tc:
        with (
            tc.tile_pool(name="consts", bufs=1) as consts,
            tc.tile_pool(name="nat", bufs=1) as nat,
            tc.tile_pool(name="stream", bufs=3) as stream,
            tc.tile_pool(name="dpool", bufs=2) as dpool,
            tc.tile_pool(name="scratch", bufs=2) as scratch,
            tc.tile_pool(name="acc", bufs=1) as accp,
            tc.tile_pool(name="psmm", bufs=2, space="PSUM") as psmm,
        ):
            ones_col = consts.tile([P, 1], f32)
            nc.vector.memset(ones_col[:], 1.0)

            small_sb = accp.tile([P, NSMALL], f32)
            nc.vector.memset(small_sb[:, 27:28], 0.0)  # unused pad col
            gw_sb = accp.tile([P, NGW], f32)

            # natural-layout block loads (row i = t*128 + p)
            zb = nat.tile([P, T_ROW, D], f32)
            pzb = nat.tile([P, T_ROW, D], f32)
            mu_t = nat.tile([P, T_ROW, D], f32)
            lv_t = nat.tile([P, T_ROW, D], f32)
            nc.sync.dma_start(out=zb[:], in_=z_blk.rearrange("(t p) d -> p t d", p=P))
            nc.sync.dma_start(out=pzb[:], in_=pz_blk.rearrange("(t p) d -> p t d", p=P))
            nc.sync.dma_start(out=mu_t[:], in_=mu_blk.rearrange("(t p) d -> p t d", p=P))
            nc.sync.dma_start(out=lv_t[:], in_=lv_blk.rearrange("(t p) d -> p t d", p=P))

            rv = r_blk.rearrange("(t p) f -> p t f", p=P)
            xv = x_blk.rearrange("(t p) f -> p t f", p=P)

            def emit_mse(k):
                t, c = divmod(k, MSE_NCH)
                lo = c * MSE_CHUNK
                rt = stream.tile([P, MSE_CHUNK], f32, tag="rt")
                xt = stream.tile([P, MSE_CHUNK], f32, tag="xt")
                nc.sync.dma_start(out=rt[:], in_=rv[:, t, lo:lo + MSE_CHUNK])
                nc.sync.dma_start(out=xt[:], in_=xv[:, t, lo:lo + MSE_CHUNK])
                dt = dpool.tile([P, MSE_CHUNK], f32, tag="dt")
                nc.vector.tensor_sub(dt[:], rt[:], xt[:])
                sc = scratch.tile([P, MSE_CHUNK], f32, tag="msq")
                nc.scalar.activation(out=sc[:], in_=dt[:], func=AF.Square,
                                     accum_out=small_sb[:, C_MSE + k:C_MSE + k + 1])

            def emit_moments(nat_t, s_col, g_lo):
                """Weighted moments of one [ROWS, D] block tensor.

                r[p,t] = |a_i|^2/2^16, e = exp(-r), S partial -> s_col,
                G = sum_i e_i a_i a_i^T and w = sum_i e_i a_i -> gw_sb cols
                [g_lo : g_lo+129].
                """
                r_t = consts.tile([P, T_ROW], f32, tag=f"r{g_lo}")
                e_t = consts.tile([P, T_ROW], f32, tag=f"e{g_lo}")
                for t in range(T_ROW):
                    sq = scratch.tile([P, D], f32, tag="momsq")
                    # Square(x/256) = x^2/65536 (scale is an exact pow2)
                    nc.scalar.activation(out=sq[:], in_=nat_t[:, t, :],
                                         func=AF.Square, scale=1.0 / 256.0,
                                         accum_out=r_t[:, t:t + 1])
                nc.scalar.activation(out=e_t[:], in_=r_t[:], func=AF.Exp,
                                     scale=-1.0, accum_out=s_col)
                sc_t = nat.tile([P, T_ROW, D], f32, tag=f"sc{g_lo}")
                for t in range(T_ROW):
                    nc.vector.tensor_scalar_mul(sc_t[:, t, :], nat_t[:, t, :],
                                                e_t[:, t:t + 1])
                ps = psmm.tile([P, GW_W], f32, tag="mom")
                for t in range(T_ROW):
                    nc.tensor.matmul(ps[:, 0:D], lhsT=sc_t[:, t, :],
                                     rhs=nat_t[:, t, :],
                                     start=(t == 0), stop=(t == T_ROW - 1))
                for t in range(T_ROW):
                    nc.tensor.matmul(ps[:, D:D + 1], lhsT=sc_t[:, t, :],
                                     rhs=ones_col[:],
                                     start=(t == 0), stop=(t == T_ROW - 1))
                nc.vector.tensor_copy(gw_sb[:, g_lo:g_lo + GW_W], ps[:])

            def emit_kld():
                nc.vector.tensor_reduce(small_sb[:, C_LV:C_LV + 1], lv_t[:],
                                        axis=AX.XY, op=ALU.add)
                k1 = scratch.tile([P, T_ROW, D], f32, tag="ksc")
                nc.scalar.activation(out=k1[:], in_=mu_t[:], func=AF.Square,
                                     accum_out=small_sb[:, C_MU2:C_MU2 + 1])
                k2 = scratch.tile([P, T_ROW, D], f32, tag="ksc")
                nc.scalar.activation(out=k2[:], in_=lv_t[:], func=AF.Exp,
                                     accum_out=small_sb[:, C_ELV:C_ELV + 1])

            for k in range(NMSE):
                emit_mse(k)
                if k == 1:
                    emit_moments(zb, small_sb[:, C_SZ:C_SZ + 1], 0)
                elif k == 3:
                    emit_moments(pzb, small_sb[:, C_SPZ:C_SPZ + 1], GW_W)
                elif k == 5:
                    emit_kld()

            nc.sync.dma_start(out=small_out, in_=small_sb[:])
            nc.sync.dma_start(out=gw_out, in_=gw_sb[:])

    nc.compile()
    return nc


def get_nc():
    if "nc" not in _CACHE:
        _CACHE["nc"] = _build()
    return _CACHE["nc"]


def make_in_maps(recons, x, z, mu, log_var, prior_z):
    r2 = np.ascontiguousarray(recons, dtype=np.float32).reshape(N, IMG_F)
    x2 = np.ascontiguousarray(x, dtype=np.float32).reshape(N, IMG_F)
    z = np.ascontiguousarray(z, dtype=np.float32)
    pz = np.ascontiguousarray(prior_z, dtype=np.float32)
    mu = np.ascontiguousarray(mu, dtype=np.float32)
    lv = np.ascontiguousarray(log_var, dtype=np.float32)
    maps = []
    for c in range(NCORES):
        s = slice(c * ROWS, (c + 1) * ROWS)
        maps.append({
            "r_blk": r2[s], "x_blk": x2[s],
            "z_blk": z[s], "pz_blk": pz[s],
            "mu_blk": mu[s], "lv_blk": lv[s],
        })
    return maps


def combine(results):
    mse_sum = 0.0
    kld_total = 0.0
    S = {"z": 0.0, "pz": 0.0}
    w = {"z": np.zeros(D), "pz": np.zeros(D)}
    G = {"z": np.zeros((D, D)), "pz": np.zeros((D, D))}
    for res in results:
        sm = np.float64(res["small_out"])
        mse_sum += sm[:, C_MSE:C_MSE + NMSE].sum()
        kld_total += (ROWS * D + sm[:, C_LV].sum() - sm[:, C_MU2].sum()
                      - sm[:, C_ELV].sum())
        S["z"] += sm[:, C_SZ].sum()
        S["pz"] += sm[:, C_SPZ].sum()
        gw = np.float64(res["gw_out"])
        G["z"] += gw[:, 0:D]
        w["z"] += gw[:, D]
        G["pz"] += gw[:, GW_W:GW_W + D]
        w["pz"] += gw[:, GW_W + D]

    def pair_sum(a, b):
        return (S[a] * S[b] + (w[a] @ w[b]) * INV_2S
                + np.sum(G[a] * G[b]) * INV_2S * INV_2S * 0.5)

    s_pp = pair_sum("pz", "pz")
    s_zz = pair_sum("z", "z")
    s_pz = pair_sum("pz", "z")

    recons_loss = mse_sum / (N * IMG_F)
    mmd = (s_pp + s_zz - 2.0 * s_pz) / (float(N) * float(N))
    kld = -0.5 * kld_total / N
    beta, alpha, reg_w = 5.0, -0.5, 100.0
    loss = (beta * recons_loss
            + (1.0 - alpha) * (1.0 / N) * kld
            + (alpha + reg_w - 1.0) / (float(N) * (N - 1)) * mmd)
    return (np.float32(loss), np.float32(recons_loss),
            np.float32(mmd), np.float32(-kld))


def run(recons, x, z, mu, log_var, prior_z, trace=False):
    from concourse.bass_utils import run_bass_kernel_spmd
    nc = get_nc()
    in_maps = make_in_maps(recons, x, z, mu, log_var, prior_z)
    res = run_bass_kernel_spmd(nc, in_maps, list(range(NCORES)), trace=trace)
    return res


def kernel(recons, x, z, mu, log_var, prior_z):
    res = run(recons, x, z, mu, log_var, prior_z)
    return combine(res.results)


# revision 5
# speedup vs baseline: 3.0404x; 1.0405x over previous
"""InfoVAE loss kernel for Trainium2, data-parallel over batch on 8 NeuronCores.

Reference computation (see problem spec):
    recons_loss = mean((recons - x)^2)                    recons/x: [4096, 3, 64, 64]
    mmd  = km(pz,pz) + km(z,z) - 2*km(pz,z)               z/pz:     [4096, 128]
           where km(a,b) = mean_ij exp(-(|a_i-b_j|^2/D)/sigma), sigma = 2*D*z_var
    kld  = mean_n(-0.5 * sum_d(1 + lv - mu^2 - exp(lv)))
    loss = 5*recons_loss + 1.5*(1/N)*kld + 98.5/(N*(N-1))*mmd
    returns (loss, recons_loss, mmd, -kld)

MMD via factored Taylor moments instead of the N^2 pairwise kernel:
    k_ij = e^{-r_i} e^{-c_j} e^{p_ij},  r_i=|a_i|^2/2^16, c_j=|b_j|^2/2^16,
    p_ij = a_i.b_j/2^15.  |p| <~ 1e-3, so e^p = 1 + p + p^2/2 to ~1e-10:
      sum_ij k = S_A*S_B + (w_A.w_B)/2^15 + <G_A, G_B>/2^31
    with per-tensor weighted moments S = sum_i e^{-r_i}, w = sum_i e^{-r_i} a_i,
    G = sum_i e^{-r_i} a_i a_i^T.  All moments are additive over row blocks, so
    each core computes them for its own 512-row block; the host sums the 8
    block moments and assembles the three pair sums in float64.  Validated vs
    the f64 ground truth: mmd rel err ~1e-7 (the fp32 jax reference itself sits
    7.2e-3 from the f64 value; tolerance is 2e-2).

This removes every N^2 term: the kernel is a pure memory-bound stream of
recons/x (50 MB/core) with the tiny moment/KLD work hidden under the DMA.
"""

import numpy as np

N = 4096
D = 128
NCORES = 8
ROWS = N // NCORES            # 512 rows per core
IMG_F = 3 * 64 * 64           # 12288
P = 128
T_ROW = ROWS // P             # 4 row tiles per core
MSE_CHUNK = 2048
MSE_NCH = IMG_F // MSE_CHUNK  # 6
NMSE = T_ROW * MSE_NCH        # 24 accum columns
INV_2S = 1.0 / 2.0 ** 15
INV_S = 1.0 / 2.0 ** 16

# small_out column map
C_MSE = 0                     # 0:24   mse partial sums
C_LV = 24                     # sum(log_var)
C_MU2 = 25                    # sum(mu^2)
C_ELV = 26                    # sum(exp(log_var))
C_SZ = 28                     # S_z partial (per-partition)
C_SPZ = 29                    # S_pz partial
NSMALL = 30
# gw_out column map: [G_z | w_z | G_pz | w_pz]
GW_W = D + 1                  # 129 columns per tensor
NGW = 2 * GW_W                # 258

_CACHE = {}


def _build():
    import concourse.bass as bass
    import concourse.tile as tile
    from concourse import bacc, mybir

    f32 = mybir.dt.float32
    AF = mybir.ActivationFunctionType
    ALU = mybir.AluOpType
    AX = mybir.AxisListType

    nc = bacc.Bacc("TRN2", target_bir_lowering=False, debug=False,
                   num_devices=NCORES)

    r_blk = nc.dram_tensor("r_blk", [ROWS, IMG_F], f32, kind="ExternalInput").ap()
    x_blk = nc.dram_tensor("x_blk", [ROWS, IMG_F], f32, kind="ExternalInput").ap()
    z_blk = nc.dram_tensor("z_blk", [ROWS, D], f32, kind="ExternalInput").ap()
    pz_blk = nc.dram_tensor("pz_blk", [ROWS, D], f32, kind="ExternalInput").ap()
    mu_blk = nc.dram_tensor("mu_blk", [ROWS, D], f32, kind="ExternalInput").ap()
    lv_blk = nc.dram_tensor("lv_blk", [ROWS, D], f32, kind="ExternalInput").ap()

    small_out = nc.dram_tensor("small_out", [P, NSMALL], f32,
                               kind="ExternalOutput").ap()
    gw_out = nc.dram_tensor("gw_out", [P, NGW], f32, kind="ExternalOutput").ap()

    with tile.TileContext(nc) as tc:
        with (
            tc.tile_pool(name="consts", bufs=1) as consts,
            tc.tile_pool(name="nat", bufs=1) as nat,
            tc.tile_pool(name="stream", bufs=4) as stream,
            tc.tile_pool(name="dpool", bufs=2) as dpool,
            tc.tile_pool(name="scratch", bufs=2) as scratch,
            tc.tile_pool(name="acc", bufs=1) as accp,
            tc.tile_pool(name="psmm", bufs=2, space="PSUM") as psmm,
        ):
            ones_col = consts.tile([P, 1], f32)
            nc.vector.memset(ones_col[:], 1.0)

            small_sb = accp.tile([P, NSMALL], f32)
            nc.vector.memset(small_sb[:, 27:28], 0.0)  # unused pad col
            gw_sb = accp.tile([P, NGW], f32)

            # small block loads on the idle SWDGE/Pool ring, row-contiguous
            # layout (row i = p*T_ROW + t -> 2KB contiguous per partition);
            # every consumer is a full-row reduction so the mapping is free.
            zb = nat.tile([P, T_ROW, D], f32)
            pzb = nat.tile([P, T_ROW, D], f32)
            mu_t = nat.tile([P, T_ROW, D], f32)
            lv_t = nat.tile([P, T_ROW, D], f32)
            nc.gpsimd.dma_start(out=zb[:], in_=z_blk.rearrange("(p t) d -> p t d", t=T_ROW))
            nc.gpsimd.dma_start(out=pzb[:], in_=pz_blk.rearrange("(p t) d -> p t d", t=T_ROW))
            nc.gpsimd.dma_start(out=mu_t[:], in_=mu_blk.rearrange("(p t) d -> p t d", t=T_ROW))
            nc.gpsimd.dma_start(out=lv_t[:], in_=lv_blk.rearrange("(p t) d -> p t d", t=T_ROW))

            rv = r_blk.rearrange("(t p) f -> p t f", p=P)
            xv = x_blk.rearrange("(t p) f -> p t f", p=P)

            def emit_mse(k):
                t, c = divmod(k, MSE_NCH)
                lo = c * MSE_CHUNK
                rt = stream.tile([P, MSE_CHUNK], f32, tag="rt")
                xt = stream.tile([P, MSE_CHUNK], f32, tag="xt")
                # split the two streams across the two HWDGE rings (SP + ACT)
                nc.sync.dma_start(out=rt[:], in_=rv[:, t, lo:lo + MSE_CHUNK])
                nc.scalar.dma_start(out=xt[:], in_=xv[:, t, lo:lo + MSE_CHUNK])
                dt = dpool.tile([P, MSE_CHUNK], f32, tag="dt")
                nc.vector.tensor_sub(dt[:], rt[:], xt[:])
                sc = scratch.tile([P, MSE_CHUNK], f32, tag="msq")
                nc.scalar.activation(out=sc[:], in_=dt[:], func=AF.Square,
                                     accum_out=small_sb[:, C_MSE + k:C_MSE + k + 1])

            def emit_moments(nat_t, s_col, g_lo):
                """Weighted moments of one [ROWS, D] block tensor.

                r[p,t] = |a_i|^2/2^16, e = exp(-r), S partial -> s_col,
                G = sum_i e_i a_i a_i^T and w = sum_i e_i a_i -> gw_sb cols
                [g_lo : g_lo+129].
                """
                r_t = consts.tile([P, T_ROW], f32, tag=f"r{g_lo}")
                e_t = consts.tile([P, T_ROW], f32, tag=f"e{g_lo}")
                for t in range(T_ROW):
                    sq = scratch.tile([P, D], f32, tag="momsq")
                    # Square(x/256) = x^2/65536 (scale is an exact pow2)
                    nc.scalar.activation(out=sq[:], in_=nat_t[:, t, :],
                                         func=AF.Square, scale=1.0 / 256.0,
                                         accum_out=r_t[:, t:t + 1])
                nc.scalar.activation(out=e_t[:], in_=r_t[:], func=AF.Exp,
                                     scale=-1.0, accum_out=s_col)
                sc_t = nat.tile([P, T_ROW, D], f32, tag=f"sc{g_lo}")
                for t in range(T_ROW):
                    nc.vector.tensor_scalar_mul(sc_t[:, t, :], nat_t[:, t, :],
                                                e_t[:, t:t + 1])
                ps = psmm.tile([P, GW_W], f32, tag="mom")
                for t in range(T_ROW):
                    nc.tensor.matmul(ps[:, 0:D], lhsT=sc_t[:, t, :],
                                     rhs=nat_t[:, t, :],
                                     start=(t == 0), stop=(t == T_ROW - 1))
                for t in range(T_ROW):
                    nc.tensor.matmul(ps[:, D:D + 1], lhsT=sc_t[:, t, :],
                                     rhs=ones_col[:],
                                     start=(t == 0), stop=(t == T_ROW - 1))
                nc.vector.tensor_copy(gw_sb[:, g_lo:g_lo + GW_W], ps[:])

            def emit_kld():
                nc.vector.tensor_reduce(small_sb[:, C_LV:C_LV + 1], lv_t[:],
                                        axis=AX.XY, op=ALU.add)
                k1 = scratch.tile([P, T_ROW, D], f32, tag="ksc")
                nc.scalar.activation(out=k1[:], in_=mu_t[:], func=AF.Square,
                                     accum_out=small_sb[:, C_MU2:C_MU2 + 1])
                k2 = scratch.tile([P, T_ROW, D], f32, tag="ksc")
                nc.scalar.activation(out=k2[:], in_=lv_t[:], func=AF.Exp,
                                     accum_out=small_sb[:, C_ELV:C_ELV + 1])

            for k in range(NMSE):
                emit_mse(k)
                if k == 2:
                    emit_moments(zb, small_sb[:, C_SZ:C_SZ + 1], 0)
                elif k == 4:
                    emit_moments(pzb, small_sb[:, C_SPZ:C_SPZ + 1], GW_W)
                elif k == 6:
                    emit_kld()
                elif k == 8:
                    nc.gpsimd.dma_start(out=gw_out, in_=gw_sb[:])

            nc.sync.dma_start(out=small_out, in_=small_sb[:])

    nc.compile()
    return nc


def get_nc():
    if "nc" not in _CACHE:
        _CACHE["nc"] = _build()
    return _CACHE["nc"]


def make_in_maps(recons, x, z, mu, log_var, prior_z):
    r2 = np.ascontiguousarray(recons, dtype=np.float32).reshape(N, IMG_F)
    x2 = np.ascontiguousarray(x, dtype=np.float32).reshape(N, IMG_F)
    z = np.ascontiguousarray(z, dtype=np.float32)
    pz = np.ascontiguousarray(prior_z, dtype=np.float32)
    mu = np.ascontiguousarray(mu, dtype=np.float32)
    lv = np.ascontiguousarray(log_var, dtype=np.float32)
    maps = []
    for c in range(NCORES):
        s = slice(c * ROWS, (c + 1) * ROWS)
        maps.append({
            "r_blk": r2[s], "x_blk": x2[s],
            "z_blk": z[s], "pz_blk": pz[s],
            "mu_blk": mu[s], "lv_blk": lv[s],
        })
    return maps


def combine(results):
    mse_sum = 0.0
    kld_total = 0.0
    S = {"z": 0.0, "pz": 0.0}
    w = {"z": np.zeros(D), "pz": np.zeros(D)}
    G = {"z": np.zeros((D, D)), "pz": np.zeros((D, D))}
    for res in results:
        sm = np.float64(res["small_out"])
        mse_sum += sm[:, C_MSE:C_MSE + NMSE].sum()
        kld_total += (ROWS * D + sm[:, C_LV].sum() - sm[:, C_MU2].sum()
                      - sm[:, C_ELV].sum())
        S["z"] += sm[:, C_SZ].sum()
        S["pz"] += sm[:, C_SPZ].sum()
        gw = np.float64(res["gw_out"])
        G["z"] += gw[:, 0:D]
        w["z"] += gw[:, D]
        G["pz"] += gw[:, GW_W:GW_W + D]
        w["pz"] += gw[:, GW_W + D]

    def pair_sum(a, b):
        return (S[a] * S[b] + (w[a] @ w[b]) * INV_2S
                + np.sum(G[a] * G[b]) * INV_2S * INV_2S * 0.5)

    s_pp = pair_sum("pz", "pz")
    s_zz = pair_sum("z", "z")
    s_pz = pair_sum("pz", "z")

    recons_loss = mse_sum / (N * IMG_F)
    mmd = (s_pp + s_zz - 2.0 * s_pz) / (float(N) * float(N))
    kld = -0.5 * kld_total / N
    beta, alpha, reg_w = 5.0, -0.5, 100.0
    loss = (beta * recons_loss
            + (1.0 - alpha) * (1.0 / N) * kld
            + (alpha + reg_w - 1.0) / (float(N) * (N - 1)) * mmd)
    return (np.float32(loss), np.float32(recons_loss),
            np.float32(mmd), np.float32(-kld))


def run(recons, x, z, mu, log_var, prior_z, trace=False):
    from concourse.bass_utils import run_bass_kernel_spmd
    nc = get_nc()
    in_maps = make_in_maps(recons, x, z, mu, log_var, prior_z)
    res = run_bass_kernel_spmd(nc, in_maps, list(range(NCORES)), trace=trace)
    return res


def kernel(recons, x, z, mu, log_var, prior_z):
    res = run(recons, x, z, mu, log_var, prior_z)
    return combine(res.results)
